# revision 25
# baseline (speedup 1.0000x reference)
"""DSH loss kernel for Trainium2 (8 NeuronCores, Bass/Tile) — v4.

Math (reference):
    U[ind] = u; Y[ind] = y
    dist[b,n]  = max(||u_b||^2 - 2 u_b.U_n + ||U_n||^2, 0)
    match[b,n] = y_b . Y_n
    loss1 = mean( (1-m)*0.5*dist + m*0.5*relu(M - dist) ),  m = (match == 0)
    loss2 = ALPHA * mean(|1 - sign(u)|)

Key decomposition (host, fp64, exact):
    2*B*N*loss1 = S_raw + sum_{match==0} [ relu(M - raw) - raw ]
    S_raw factorizes (N*sum|u|^2 + B*sum|U|^2 - 2 colsum(u).colsum(U)) and the
    correction only needs the match==0 pairs, which are astronomically rare
    for random labels. The device is purely a *detector* of match==0 pairs;
    flagged cells are re-checked and corrected exactly on host.

Device detector (per core, shard = 12500 gallery rows -> 25 chunks x 512):
  * Batch AND-compression (host): group labels g_i = y_{2i} & y_{2i+1}
    (pointwise product of binary labels). Since g <= y pointwise and labels
    are non-negative, g.Y_n <= y_b.Y_n, so every true mismatch still yields
    g.Y_n == 0 -> NO false negatives, ever. False positives ((7/8)^100 per
    pair ~ 1.6e-6) are rare and re-checked on host. Halves matmul + scan work.
  * One fp8(e4m3) DoubleRow matmul per (chunk, group-half): labels split in
    two k-tiles of 50 classes on 50 partitions; {0,1} values are exact in
    fp8 and PSUM accumulates in fp32, so match values are EXACT integers.
    DoubleRow runs at 0.5 cycles/row -> ~2x bf16 matmul throughput, and the
    gallery operand is only 1.28 MB/core of DMA.
  * Zero-detection scan, statically split between two engines:
      - DVE: tensor_tensor_reduce product of two PSUM chunk-tiles
        (x1_cA * x1_cB, min-accum): product==0 iff either match==0; reads
        2 PSUM values/cycle -> half-rate scan cost per element.
      - ScalarE: activation relu(0.5 - x1) with accum_out (sum): positive
        iff some match==0 in the chunk.
    accD[128, n_entries]: one column per scan entry; host decodes via the
    same static schedule.
"""

import numpy as np
import ml_dtypes

import concourse.bass as bass
import concourse.mybir as mybir
import concourse.tile as tile
from concourse import bacc
from concourse.bass_utils import run_bass_kernel_spmd

# Problem constants (hardcoded per harness contract)
B = 512
BIT = 64
C = 100
N = 100000
N_CORES = 8
N_SH = N // N_CORES          # 12500
M_MARGIN = 2.0 * BIT         # 128.0
ALPHA = 0.1
G = B // 2                   # 256 AND-compressed groups
KPAD = 128                   # label rows padded to full 128 partitions
F_CHUNK = 512                # gallery columns per MM (1 PSUM bank out)

FP8 = ml_dtypes.float8_e4m3
BF16 = ml_dtypes.bfloat16


def _schedule(n_chunks: int):
    """Static EW schedule over chunks (chunk = [128, 1024] PSUM = 2 banks,
    a rotating pool of 4 such tiles so each engine is double-buffered and
    matmul refill overlaps the partner engine's scan):
      ("D",  c) = DVE tensor_reduce min over chunk c    (flag: col < 0.5)
      ("A1", c) = ScalarE relu+accum over chunk c       (flag: col > 0.25)
    GpSimd cannot access PSUM on TRN2 (BIR-verified), and dual-PSUM-input
    DVE ops are also rejected, so exactly these two single-source scans are
    available. A wider shared ring (v5) serializes refills against scans --
    measured Act pitch 3.3us vs 2.05us busy -- so plain alternation wins.
    The last (partial) chunk is one DVE 3D reduce over both group halves,
    scanning only real columns: ("DX", c)."""
    sched = [("D" if c % 2 == 0 else "A1", c) for c in range(n_chunks - 1)]
    sched.append(("DX", n_chunks - 1))
    return sched


def _build_program(n_sh: int):
    fp32 = mybir.dt.float32
    bf16 = mybir.dt.bfloat16
    fp8 = mybir.dt.float8e4
    nc = bacc.Bacc("TRN2", target_bir_lowering=False)

    n_pad = ((n_sh + F_CHUNK - 1) // F_CHUNK) * F_CHUNK
    n_chunks = n_pad // F_CHUNK
    sched = _schedule(n_chunks)
    n_cols = len(sched)

    gT_d = nc.declare_dram_parameter("gT", [KPAD, G], fp8, isOutput=False)
    YT_d = nc.declare_dram_parameter("YT", [KPAD, n_pad], fp8, isOutput=False)
    accD_d = nc.declare_dram_parameter("accD", [128, n_cols], fp32, isOutput=True)

    with tile.TileContext(nc) as tc:
        with (
            tc.tile_pool(name="resident", bufs=1) as resident,
            tc.tile_pool(name="scr", bufs=4) as scrp,
            tc.tile_pool(name="psum", bufs=4, space="PSUM") as psump,
        ):
            gT = resident.tile([KPAD, G], fp8, tag="gT")
            YT = resident.tile([KPAD, n_pad], fp8, tag="YT")
            accD = resident.tile([128, n_cols], fp32, tag="accD")
            bias05 = resident.tile([128, 1], fp32, tag="bias05")
            warm = resident.tile([KPAD, F_CHUNK], fp8, tag="warm")

            # gT + first slice (tiny; needed by the first matmuls) on the
            # scalar queue; remaining gallery slices alternate sync/gpsimd
            # with ascending widths so chunk 0 is ready ASAP while later
            # slices amortize DMA issue cost.
            nc.scalar.dma_start(gT[:], gT_d[:])
            widths = [512, 512, 1024, 1024, 2048, 2048, 2816, 2816]
            s = 0
            slices = []
            for w in widths:
                if s >= n_pad:
                    break
                w = min(w, n_pad - s)
                slices.append((s, s + w))
                s += w
            while s < n_pad:
                w = min(2816, n_pad - s)
                slices.append((s, s + w))
                s += w
            # slice 0 leads the sync queue so the first matmul unblocks ASAP
            for i, (a, b) in enumerate(slices):
                q = nc.sync if i % 2 == 0 else nc.gpsimd
                q.dma_start(YT[:, a:b], YT_d[:, a:b])

            nc.vector.memset(bias05[:], 0.5)
            nc.vector.memset(warm[:], 0.0)

            # p-state warmup: the PE only reaches its top clock after ~3us of
            # continuous execution; these dummy matmuls (inputs: the zeroed
            # warm tile, output overwritten by the real stream) run during
            # the DMA prologue so the real stream starts at full speed.
            warm_out = psump.tile([128, 2 * F_CHUNK], fp32, tag="x1")
            for _ in range(3):
                nc.tensor.matmul(
                    warm_out[:, :F_CHUNK],
                    lhsT=warm[:, :128], rhs=warm[:],
                    start=True, stop=True,
                )

            # real columns in the last (partial) chunk -- pad cols skipped
            n_last = n_sh - (n_chunks - 1) * F_CHUNK
            for i, ent in enumerate(sched):
                col = accD[:, i : i + 1]
                c = ent[1]
                x1 = psump.tile([128, 2 * F_CHUNK], fp32, tag="x1")
                for si in (0, 1):
                    nc.tensor.matmul(
                        x1[:, si * F_CHUNK : (si + 1) * F_CHUNK],
                        lhsT=gT[:, si * 128 : (si + 1) * 128],
                        rhs=YT[:, c * F_CHUNK : (c + 1) * F_CHUNK],
                        start=True, stop=True,
                    )
                if ent[0] == "D":
                    nc.vector.tensor_reduce(
                        col, x1[:],
                        mybir.AxisListType.X, mybir.AluOpType.min,
                    )
                elif ent[0] == "A1":
                    scr = scrp.tile([128, 1024], bf16, tag="scrA1")
                    nc.scalar.activation(
                        scr[:], x1[:],
                        mybir.ActivationFunctionType.Relu,
                        bias=bias05[:], scale=-1.0,
                        accum_out=col,
                    )
                else:  # "DX": both group halves of the last chunk, real cols
                    x3 = x1[:].rearrange("p (h f) -> p h f", h=2)
                    nc.vector.tensor_reduce(
                        col, x3[:, :, :n_last],
                        mybir.AxisListType.XY, mybir.AluOpType.min,
                    )

            # split the output DMA so most of it overlaps the scan stream
            cut = max(0, n_cols - 3)
            if cut:
                nc.sync.dma_start(accD_d[:, :cut], accD[:, :cut])
            nc.sync.dma_start(accD_d[:, cut:], accD[:, cut:])

    nc.finalize()
    return nc, sched


def _prep_host(u, y, ind, U, Y):
    """Scatter + device fp8 arrays + fp64 base sum."""
    u = np.asarray(u, dtype=np.float32)
    y = np.asarray(y, dtype=np.float32)
    ind = np.asarray(ind).astype(np.int64)
    U2 = np.array(U, dtype=np.float32, copy=True)
    Y2 = np.array(Y, dtype=np.float32, copy=True)
    U2[ind] = u
    Y2[ind] = y

    u64 = u.astype(np.float64)
    U64 = U2.astype(np.float64)
    u_sq64 = (u64 * u64).sum(axis=1)            # [B]
    U_sq64 = (U64 * U64).sum(axis=1)            # [N]
    s_raw = (
        N * u_sq64.sum()
        + B * U_sq64.sum()
        - 2.0 * (u64.sum(axis=0) @ U64.sum(axis=0))
    )

    # AND-compressed group labels: g_i = y_{2i} * y_{2i+1}  [G, C]
    g = (y[0::2] * y[1::2])
    gT = np.zeros((KPAD, G), dtype=FP8)
    gT[:C] = g.T.astype(FP8)                    # rows C..KPAD stay 0

    n_pad = ((N_SH + F_CHUNK - 1) // F_CHUNK) * F_CHUNK
    # YT per core, [KPAD, n_pad]; pad columns all-ones in the label rows
    # (match every group with |g|>=1 -> never flagged; host precondition
    # guarantees |g|>=1). Rows C..KPAD are zero everywhere.
    YTs = []
    for c in range(N_CORES):
        sl = Y2[c * N_SH : (c + 1) * N_SH]      # [N_SH, C]
        yt = np.zeros((KPAD, n_pad), dtype=FP8)
        yt[:C, :N_SH] = sl.T.astype(FP8)
        yt[:C, N_SH:] = FP8(1.0)
        YTs.append(yt)

    return u, y, U2, Y2, gT, YTs, s_raw


def _full_numpy_loss(u, y, U2, Y2):
    """Exact fp64 fallback (blocked); only used if detector preconditions
    fail (non-binary labels / empty groups) -- never on spec inputs."""
    total = 0.0
    U64 = U2.astype(np.float64)
    Y64 = Y2.astype(np.float64)
    U_sq = (U64 * U64).sum(axis=1)
    for b0 in range(0, B, 64):
        ub = u[b0 : b0 + 64].astype(np.float64)
        yb = y[b0 : b0 + 64].astype(np.float64)
        dist = np.maximum(
            (ub * ub).sum(1)[:, None] - 2.0 * (ub @ U64.T) + U_sq[None, :], 0.0)
        mism = (yb @ Y64.T) == 0.0
        total += np.where(mism, 0.5 * np.maximum(M_MARGIN - dist, 0.0),
                          0.5 * dist).sum()
    loss1 = total / (B * N)
    loss2 = ALPHA * np.abs(1.0 - np.sign(u)).mean(dtype=np.float64)
    return np.array(loss1 + loss2, dtype=np.float32)


def _detector_preconditions_ok(y, Y2):
    if not (((y == 0.0) | (y == 1.0)).all() and ((Y2 == 0.0) | (Y2 == 1.0)).all()):
        return False
    # every AND-group must be non-empty, else pad columns could flag
    g = y[0::2] * y[1::2]
    return bool((g.sum(axis=1) >= 1.0).all())


_PROG_CACHE = {}


def _get_program():
    key = ("v4", N_SH)
    if key not in _PROG_CACHE:
        _PROG_CACHE[key] = _build_program(N_SH)
    return _PROG_CACHE[key]


def kernel(u, y, ind, U, Y):
    u, y, U2, Y2, gT, YTs, s_raw = _prep_host(u, y, ind, U, Y)

    if not _detector_preconditions_ok(y, Y2):
        return _full_numpy_loss(u, y, U2, Y2)

    nc, sched = _get_program()
    in_maps = [{"gT": gT, "YT": YTs[c]} for c in range(N_CORES)]

    res = run_bass_kernel_spmd(nc, in_maps, list(range(N_CORES)))
    results = res.results

    y64 = y.astype(np.float64)
    Y64 = Y2.astype(np.float64)
    u64 = u.astype(np.float64)
    U64 = U2.astype(np.float64)

    # safety valve: absurd flag counts mean something unexpected about the
    # inputs (e.g. near-empty label sets) -- recompute exactly instead of
    # iterating millions of host rechecks
    def _flag_rows(accD, i, ent):
        if ent[0] in ("D", "DX"):
            return np.nonzero(accD[:, i] < 0.5)[0]
        return np.nonzero(accD[:, i] > 0.25)[0]   # A1

    total_flags = 0
    for core in range(N_CORES):
        accD = np.asarray(results[core]["accD"], dtype=np.float64)
        for i, ent in enumerate(sched):
            total_flags += len(_flag_rows(accD, i, ent))
    if total_flags > 20000:
        return _full_numpy_loss(u, y, U2, Y2)

    corr = 0.0
    for core in range(N_CORES):
        accD = np.asarray(results[core]["accD"], dtype=np.float64)
        for i, ent in enumerate(sched):
            ch = ent[1]
            nch = 2 if ent[0] == "A2" else 1
            lo = ch * F_CHUNK
            hi = min((ch + nch) * F_CHUNK, N_SH)
            if lo >= hi:
                continue
            for p in _flag_rows(accD, i, ent):
                rows = [2 * p, 2 * p + 1, 2 * (p + 128), 2 * (p + 128) + 1]
                n_glob = np.arange(core * N_SH + lo, core * N_SH + hi)
                match = y64[rows] @ Y64[n_glob].T            # [4, hi-lo]
                bz, nz = np.nonzero(match == 0.0)
                for b_i, n_i in zip(bz, nz):
                    b = rows[b_i]
                    n = n_glob[n_i]
                    d = u64[b] - U64[n]
                    raw = float(d @ d)
                    corr += max(M_MARGIN - raw, 0.0) - raw

    total2 = s_raw + corr
    loss1 = 0.5 * total2 / (B * N)

    loss2 = ALPHA * np.abs(1.0 - np.sign(u)).mean(dtype=np.float64)

    return np.array(loss1 + loss2, dtype=np.float32)
